# revision 6
# baseline (speedup 1.0000x reference)
"""Trainium2 Bass kernel for DropChannel (topk channel masking).

Math (per sample):
    score_c = mean_hw x[hw, c]                       (only sums needed; 1/HW cancels)
    lk_c    = ln(r_c) * (1 / S_c)                    (log of key r**(1/score); order-preserving)
    gcnt_i  = #{c : lk_c > lk_i}                     (strictly-greater count)
    sel_i   = gcnt_i < C - M                         (identical to thr = sort(key)[C-M]; sel = key >= thr,
                                                      including tie behaviour)
    alpha   = sum(S) / sum(S * sel)
    out     = x * (sel & (u < P)) * alpha

Sharding: pure data parallel, N=32 samples -> 8 cores x 4 samples.

Per-core schedule (4 samples, each [4096, 1024] f32):
  - x repacked so each partition holds FOUR consecutive hw rows contiguous
    in HBM -> 16 KiB DMA descriptors (4x fewer packets than row-per-
    partition layout, amortizing per-packet overhead)
  - column sums via fp32 PE matmuls accumulating into two PSUM parity
    slots (16-deep accumulation each, matching the fp32 precision profile
    the selection boundary needs), summed at the end
  - score transposed to per-partition layout with 8 tiny PE matmuls
    (lhsT = s_row slice, rhs = [1,1] ones) instead of a queued DMA hop,
    keeping the mask chain off the DMA queues entirely
  - greater-counts: 2x-mode DVE compares of broadcast-lk against
    per-partition lk scalars, summed across partitions by full-rate bf16
    PE ones-matmuls (exact 0/1 integer arithmetic)
  - mask rows replicated via gpsimd.partition_broadcast; in-place multiply
    of cached chunks; loads ride qSP, stores ride qACT
"""

import numpy as np
from contextlib import ExitStack

import concourse.bacc as bacc
import concourse.tile as tile
from concourse import mybir
from concourse.bass_utils import run_bass_kernel_spmd

N, HW, C = 32, 4096, 1024
NCORES = 8
NS = N // NCORES          # samples per core
P = 128                   # partitions
CK = C // P               # 8 channels per partition in (k p) layout
NKEEP = C - int(0.5 * C)  # gcnt threshold: keep rows with gcnt < 512
PKEEP = 0.9
HALF = 512                # matmul free-dim limit (one PSUM bank)
FOUR = 4                  # hw rows per partition per chunk (16 KiB descriptors)
CHF = FOUR * C            # chunk free dim (f32 elems per partition)

f32 = mybir.dt.float32
bf16 = mybir.dt.bfloat16
ALU = mybir.AluOpType
ACTF = mybir.ActivationFunctionType
AXIS = mybir.AxisListType


def emit(tc, o, x, r, u, ns, hw, xbufs):
    nc = tc.nc
    nch = hw // (P * FOUR)                # chunks per sample
    xq = x.rearrange("s (q p four) c -> s q p (four c)", p=P, four=FOUR)
    oq = o.rearrange("s (q p four) c -> s q p (four c)", p=P, four=FOUR)
    rkp = r.rearrange("s (k p) -> s p k", p=P)   # channel c = k*128 + p

    with ExitStack() as ctx:
        xpool = ctx.enter_context(tc.tile_pool(name="xpool", bufs=xbufs))
        tqpool = ctx.enter_context(tc.tile_pool(name="tqpool", bufs=3))
        bcpool = ctx.enter_context(tc.tile_pool(name="bcpool", bufs=1))
        rows = ctx.enter_context(tc.tile_pool(name="rows", bufs=1))
        consts = ctx.enter_context(tc.tile_pool(name="consts", bufs=1))
        ps_s = ctx.enter_context(tc.tile_pool(name="ps_s", bufs=1, space="PSUM"))
        ps_g = ctx.enter_context(tc.tile_pool(name="ps_g", bufs=1, space="PSUM"))
        ps_tp = ctx.enter_context(tc.tile_pool(name="ps_tp", bufs=1, space="PSUM"))

        ones_col = consts.tile([P, 1], f32)
        nc.vector.memset(ones_col, 1.0)
        ones_b = consts.tile([P, 1], bf16)
        nc.vector.memset(ones_b, 1.0)
        one_one = consts.tile([1, 1], f32)
        nc.vector.memset(one_one, 1.0)

        for s in range(ns):
            # precompute pieces that do not depend on x (overlap with loads):
            # ln(r) in both layouts, and the bernoulli gate row (u < PKEEP)
            lnr_cols = rows.tile([P, CK], f32, tag="lnr_cols")
            nc.scalar.dma_start(out=lnr_cols, in_=rkp[s])
            nc.scalar.activation(lnr_cols, lnr_cols, ACTF.Ln)
            lnr_row = rows.tile([1, C], f32, tag="lnr_row")
            nc.scalar.dma_start(out=lnr_row, in_=r[s:s + 1, :])
            nc.scalar.activation(lnr_row, lnr_row, ACTF.Ln)
            rng_row = rows.tile([1, C], f32, tag="rng_row")
            nc.scalar.dma_start(out=rng_row, in_=u[s:s + 1, :])
            nc.gpsimd.tensor_scalar(rng_row, rng_row, PKEEP, None, op0=ALU.is_lt)

            # ---- pass 1: chunk loads + score matmuls (two PSUM parity
            # slots of 16 accumulations each, keeping fp32 rounding at the
            # baseline's level; selection sits ~1e-5 from the boundary)
            ps_score = ps_s.tile([1, 2 * C], f32, tag="ps_score")
            xts = []
            for t in range(nch):
                xtile = xpool.tile([P, CHF], f32, tag="xt")
                # sample 0 has no store traffic yet: split its loads across
                # both HWDGE queues
                if s == 0 and t % 2 == 1:
                    nc.scalar.dma_start(out=xtile, in_=xq[s, t])
                else:
                    nc.sync.dma_start(out=xtile, in_=xq[s, t])
                xts.append(xtile)
                j = t % 2
                for f in range(FOUR):
                    for h in range(2):
                        nc.tensor.matmul(
                            ps_score[:, j * C + h * HALF:j * C + (h + 1) * HALF],
                            lhsT=ones_col,
                            rhs=xtile[:, f * C + h * HALF:f * C + (h + 1) * HALF],
                            start=(t < 2 and f == 0),
                            stop=(t >= nch - 2 and f == FOUR - 1),
                        )

            # ---- mid: selection mask ----
            # parity-slot add: only one PSUM operand allowed per DVE op, so
            # stage slot 1 through SBUF (lk_row is free scratch here — it is
            # overwritten by the reciprocal below)
            s_row = rows.tile([1, C], f32, tag="s_row")
            lk_row = rows.tile([1, C], f32, tag="lk_row")
            nc.scalar.copy(lk_row, ps_score[:, C:2 * C])
            nc.vector.tensor_add(s_row, ps_score[:, 0:C], lk_row)
            # transpose score to per-partition layout on the PE: 8 tiny
            # matmuls (lhsT = 128-wide slice of s_row, rhs = [1,1] ones)
            ps_t = ps_tp.tile([P, CK], f32, tag="ps_t")
            for i in range(CK):
                nc.tensor.matmul(
                    ps_t[:, i:i + 1],
                    lhsT=s_row[:, i * P:(i + 1) * P],
                    rhs=one_one,
                    start=True,
                    stop=True,
                )
            recip_cols = rows.tile([P, CK], f32, tag="recip_cols")
            nc.vector.reciprocal(recip_cols, ps_t)
            lk_cols = rows.tile([P, CK], f32, tag="lk_cols")
            nc.vector.tensor_mul(lk_cols, lnr_cols, recip_cols)
            # row-layout lk + broadcast (gpsimd), in parallel with col path
            nc.vector.reciprocal(lk_row, s_row)
            nc.vector.tensor_mul(lk_row, lnr_row, lk_row)
            b_bc = bcpool.tile([P, C], f32, tag="b_bc")
            stats = rows.tile([1, 4], f32, tag="stats")
            mask_row = rows.tile([1, C], f32, tag="mask_row")
            nc.gpsimd.partition_broadcast(b_bc[:, 0:HALF], lk_row[:, 0:HALF])
            nc.gpsimd.partition_broadcast(b_bc[:, HALF:], lk_row[:, HALF:])
            # total score sum for alpha (scalar engine, off the DVE critical
            # path; mask_row is pure scratch output here, overwritten below)
            nc.scalar.activation(
                mask_row, s_row, ACTF.Identity, accum_out=stats[:, 1:2]
            )
            # gcnt_row[i] = #{c : lk_c > lk_i}: 2x-mode compares feeding
            # full-rate bf16 ones-matmuls (0/1 values are exact in bf16)
            ps_gcnt = ps_g.tile([1, C], f32, tag="ps_gcnt")
            mask_bc = bcpool.tile([P, C], f32, tag="mask_bc")
            for h in range(2):
                sl = slice(h * HALF, (h + 1) * HALF)
                for q in range(CK):
                    tq = tqpool.tile([P, HALF], bf16, tag="tq")
                    nc.vector.tensor_scalar(
                        tq, b_bc[:, sl], lk_cols[:, q:q + 1], None, op0=ALU.is_lt
                    )
                    nc.tensor.matmul(
                        ps_gcnt[:, sl],
                        lhsT=ones_b,
                        rhs=tq,
                        start=(q == 0),
                        stop=(q == CK - 1),
                    )
                nc.vector.scalar_tensor_tensor(
                    mask_row[:, sl], ps_gcnt[:, sl], float(NKEEP), rng_row[:, sl],
                    op0=ALU.is_lt, op1=ALU.mult,
                )
                nc.gpsimd.partition_broadcast(mask_bc[:, sl], mask_row[:, sl])

            # alpha = sum(S) / sum(S * sel); rng_row doubles as scratch out
            nc.vector.scalar_tensor_tensor(
                rng_row, ps_gcnt, float(NKEEP), s_row,
                op0=ALU.is_lt, op1=ALU.mult, accum_out=stats[:, 0:1],
            )
            nc.vector.reciprocal(stats[:, 2:3], stats[:, 0:1])
            nc.vector.tensor_scalar(
                stats[:, 2:3], stats[:, 2:3], stats[:, 1:2], None, op0=ALU.mult
            )
            alpha_pp = rows.tile([P, 1], f32, tag="alpha_pp")
            nc.gpsimd.partition_broadcast(alpha_pp, stats[:, 2:3])

            # ---- pass 2: out = (x * alpha) * mask, in place, then store ----
            for t in range(nch):
                for f in range(FOUR):
                    nc.vector.scalar_tensor_tensor(
                        xts[t][:, f * C:(f + 1) * C],
                        xts[t][:, f * C:(f + 1) * C],
                        alpha_pp, mask_bc,
                        op0=ALU.mult, op1=ALU.mult,
                    )
                nc.scalar.dma_start(out=oq[s, t], in_=xts[t])


def build_nc(ns=NS, hw=HW, xbufs=11):
    nc = bacc.Bacc(
        "TRN2", target_bir_lowering=False, debug=False, num_devices=NCORES
    )
    x = nc.dram_tensor("x", [ns, hw, C], f32, kind="ExternalInput").ap()
    r = nc.dram_tensor("r", [ns, C], f32, kind="ExternalInput").ap()
    u = nc.dram_tensor("u", [ns, C], f32, kind="ExternalInput").ap()
    o = nc.dram_tensor("o", [ns, hw, C], f32, kind="ExternalOutput").ap()
    with tile.TileContext(nc) as tc:
        emit(tc, o, x, r, u, ns, hw, xbufs)
    nc.compile()
    return nc


_cached_nc = None


def kernel(x, r, u):
    global _cached_nc
    if _cached_nc is None:
        _cached_nc = build_nc()
    in_maps = [
        {
            "x": np.ascontiguousarray(x[i * NS:(i + 1) * NS], dtype=np.float32),
            "r": np.ascontiguousarray(r[i * NS:(i + 1) * NS], dtype=np.float32),
            "u": np.ascontiguousarray(u[i * NS:(i + 1) * NS], dtype=np.float32),
        }
        for i in range(NCORES)
    ]
    res = run_bass_kernel_spmd(_cached_nc, in_maps, list(range(NCORES))).results
    return np.concatenate([res[i]["o"] for i in range(NCORES)], axis=0)


# revision 13
# speedup vs baseline: 1.0117x; 1.0117x over previous
"""Trainium2 Bass kernel for DropChannel (topk channel masking).

Math (per sample):
    score_c = mean_hw x[hw, c]                       (only sums needed; 1/HW cancels)
    lk_c    = ln(r_c) * (1 / S_c)                    (log of key r**(1/score); order-preserving)
    gcnt_i  = #{c : lk_c > lk_i}                     (strictly-greater count)
    sel_i   = gcnt_i < C - M                         (identical to thr = sort(key)[C-M]; sel = key >= thr,
                                                      including tie behaviour)
    alpha   = sum(S) / sum(S * sel)
    out     = x * (sel & (u < P)) * alpha

Sharding: pure data parallel, N=32 samples -> 8 cores x 4 samples.

Per-core schedule (4 samples, each [4096, 1024] f32):
  - x repacked so each partition holds FOUR consecutive hw rows contiguous
    in HBM -> 16 KiB DMA descriptors (4x fewer packets, measured ~26 B/ns
    per DMA engine vs ~20 at 4 KiB)
  - per chunk, a 2-level f-slice tree-add on DVE (3 adds) feeds 2 fp32 PE
    matmuls into a single 8-deep PSUM slot: fp32 matmuls cost ~1030ns per
    512 cols, so tree-adds keep PE off the critical path and the rounding
    profile at/below the baseline's (selection sits ~1e-5 from the
    boundary, so score error must stay ~1e-6)
  - score/lk layout transposes via small DMA hops (32B descriptors, cheap);
    reciprocal only ever runs on the [128, 8] column layout (a
    single-partition [1,C] DVE reciprocal costs ~7.8us)
  - lk broadcast b_bc built by PE K=1 ones-matmuls straight into PSUM
  - greater-counts: DVE compares of b_bc against per-partition lk scalars,
    summed across partitions by full-rate bf16 PE ones-matmuls
  - mask rows (0/1, exact in bf16) replicated via gpsimd; in-place multiply
    of cached chunks; loads ride qSP, stores ride qACT
"""

import numpy as np
from contextlib import ExitStack

import concourse.bacc as bacc
import concourse.tile as tile
from concourse import mybir
from concourse.bass_utils import run_bass_kernel_spmd

N, HW, C = 32, 4096, 1024
NCORES = 8
NS = N // NCORES          # samples per core
P = 128                   # partitions
CK = C // P               # 8 channels per partition in (k p) layout
NKEEP = C - int(0.5 * C)  # gcnt threshold: keep rows with gcnt < 512
PKEEP = 0.9
HALF = 512                # matmul free-dim limit (one PSUM bank)
FOUR = 4                  # hw rows per partition per chunk (16 KiB descriptors)
CHF = FOUR * C            # chunk free dim (f32 elems per partition)

f32 = mybir.dt.float32
bf16 = mybir.dt.bfloat16
ALU = mybir.AluOpType
ACTF = mybir.ActivationFunctionType
AXIS = mybir.AxisListType


def emit(tc, o, x, r, u, ns, hw, xbufs):
    nc = tc.nc
    nch = hw // (P * FOUR)                # chunks per sample
    xq = x.rearrange("s (q p four) c -> s q p (four c)", p=P, four=FOUR)
    oq = o.rearrange("s (q p four) c -> s q p (four c)", p=P, four=FOUR)
    rck = r.rearrange("s (p k) -> s p k", k=CK)  # channel c = p*8 + k

    with ExitStack() as ctx:
        xpool = ctx.enter_context(tc.tile_pool(name="xpool", bufs=xbufs))
        accpool = ctx.enter_context(tc.tile_pool(name="accpool", bufs=2))
        tqpool = ctx.enter_context(tc.tile_pool(name="tqpool", bufs=2))
        bcpool = ctx.enter_context(tc.tile_pool(name="bcpool", bufs=1))
        rows = ctx.enter_context(tc.tile_pool(name="rows", bufs=1))
        consts = ctx.enter_context(tc.tile_pool(name="consts", bufs=1))
        ps_s = ctx.enter_context(tc.tile_pool(name="ps_s", bufs=1, space="PSUM"))
        ps_g = ctx.enter_context(tc.tile_pool(name="ps_g", bufs=1, space="PSUM"))
        ps_bc = ctx.enter_context(tc.tile_pool(name="ps_bc", bufs=1, space="PSUM"))

        ones_col = consts.tile([P, 1], f32)
        nc.vector.memset(ones_col, 1.0)
        ones_b = consts.tile([P, 1], bf16)
        nc.vector.memset(ones_b, 1.0)
        ones_row = consts.tile([1, P], f32)
        nc.vector.memset(ones_row, 1.0)

        for s in range(ns):
            # precompute pieces that do not depend on x (overlap with loads)
            lnr_cols = rows.tile([P, CK], f32, tag="lnr_cols")
            nc.scalar.dma_start(out=lnr_cols, in_=rck[s])
            nc.scalar.activation(lnr_cols, lnr_cols, ACTF.Ln)
            rng_row = rows.tile([1, C], f32, tag="rng_row")
            nc.scalar.dma_start(out=rng_row, in_=u[s:s + 1, :])
            nc.vector.tensor_scalar(rng_row, rng_row, PKEEP, None, op0=ALU.is_lt)

            # ---- pass 1: chunk loads; f-slice tree-add; 2 matmuls/chunk
            ps_score = ps_s.tile([1, C], f32, tag="ps_score")
            xts = []
            for t in range(nch):
                xtile = xpool.tile([P, CHF], f32, tag="xt")
                # sample 0 has no store traffic yet: split its loads across
                # both HWDGE queues
                if s == 0 and t % 2 == 1:
                    nc.scalar.dma_start(out=xtile, in_=xq[s, t])
                else:
                    nc.sync.dma_start(out=xtile, in_=xq[s, t])
                xts.append(xtile)
                t0 = accpool.tile([P, C], f32, tag="t0", bufs=2)
                t1 = accpool.tile([P, C], f32, tag="t1", bufs=1)
                nc.vector.tensor_add(t0, xtile[:, 0:C], xtile[:, C:2 * C])
                nc.vector.tensor_add(t1, xtile[:, 2 * C:3 * C], xtile[:, 3 * C:])
                nc.vector.tensor_add(t0, t0, t1)
                for h in range(2):
                    nc.tensor.matmul(
                        ps_score[:, h * HALF:(h + 1) * HALF],
                        lhsT=ones_col,
                        rhs=t0[:, h * HALF:(h + 1) * HALF],
                        start=(t == 0),
                        stop=(t == nch - 1),
                    )

            # ---- mid: selection mask ----
            # PSUM -> SBUF score row, halves copied by both engines at once
            s_row = rows.tile([1, C], f32, tag="s_row")
            nc.scalar.copy(s_row[:, 0:HALF], ps_score[:, 0:HALF])
            nc.vector.tensor_copy(s_row[:, HALF:], ps_score[:, HALF:])
            # column layout via DMA hop; reciprocal only ever runs on the
            # [128, 8] layout (a [1, C] DVE reciprocal costs ~7.8us)
            s_cols = rows.tile([P, CK], f32, tag="s_cols")
            nc.scalar.dma_start(out=s_cols, in_=s_row)
            # total score sum for alpha (scalar engine, off the DVE critical
            # path; mask_row is pure scratch output here, overwritten below)
            stats = rows.tile([1, 4], f32, tag="stats")
            mask_row = rows.tile([1, C], bf16, tag="mask_row")
            nc.scalar.activation(
                mask_row, s_row, ACTF.Identity, accum_out=stats[:, 1:2]
            )
            recip_cols = rows.tile([P, CK], f32, tag="recip_cols")
            nc.vector.reciprocal(recip_cols, s_cols)
            lk_cols = rows.tile([P, CK], f32, tag="lk_cols")
            nc.vector.tensor_mul(lk_cols, lnr_cols, recip_cols)
            # gcnt_row[i] = #{c : lk_c > lk_i}: DVE compares of the PE-
            # broadcast lk row against per-partition lk scalars, summed
            # across partitions by full-rate bf16 PE ones-matmuls (0/1
            # values are exact in bf16). Pipelined in 512-channel halves.
            lk_row = rows.tile([1, C], f32, tag="lk_row")
            b_bc = ps_bc.tile([P, C], f32, tag="b_bc")
            ps_gcnt = ps_g.tile([1, C], f32, tag="ps_gcnt")
            mask_bc = bcpool.tile([P, C], bf16, tag="mask_bc")
            for h in range(2):
                sl = slice(h * HALF, (h + 1) * HALF)
                nc.scalar.dma_start(
                    out=lk_row[:, sl], in_=lk_cols[h * 64:(h + 1) * 64, :]
                )
                nc.tensor.matmul(
                    b_bc[:, sl],
                    lhsT=ones_row,
                    rhs=lk_row[:, sl],
                    start=True,
                    stop=True,
                )
                for q in range(CK):
                    tq = tqpool.tile([P, HALF], bf16, tag="tq")
                    nc.vector.tensor_scalar(
                        tq, b_bc[:, sl], lk_cols[:, q:q + 1], None, op0=ALU.is_lt
                    )
                    nc.tensor.matmul(
                        ps_gcnt[:, sl],
                        lhsT=ones_b,
                        rhs=tq,
                        start=(q == 0),
                        stop=(q == CK - 1),
                    )
                nc.vector.scalar_tensor_tensor(
                    mask_row[:, sl], ps_gcnt[:, sl], float(NKEEP), rng_row[:, sl],
                    op0=ALU.is_lt, op1=ALU.mult,
                )
                nc.gpsimd.partition_broadcast(mask_bc[:, sl], mask_row[:, sl])

            # alpha = sum(S) / sum(S * sel); rng_row doubles as scratch out
            nc.vector.scalar_tensor_tensor(
                rng_row, ps_gcnt, float(NKEEP), s_row,
                op0=ALU.is_lt, op1=ALU.mult, accum_out=stats[:, 0:1],
            )
            nc.vector.reciprocal(stats[:, 2:3], stats[:, 0:1])
            nc.vector.tensor_scalar(
                stats[:, 2:3], stats[:, 2:3], stats[:, 1:2], None, op0=ALU.mult
            )
            alpha_pp = rows.tile([P, 1], f32, tag="alpha_pp")
            nc.gpsimd.partition_broadcast(alpha_pp, stats[:, 2:3])

            # ---- pass 2: out = (x * alpha) * mask, in place, then store ----
            for t in range(nch):
                for f in range(FOUR):
                    nc.vector.scalar_tensor_tensor(
                        xts[t][:, f * C:(f + 1) * C],
                        xts[t][:, f * C:(f + 1) * C],
                        alpha_pp, mask_bc,
                        op0=ALU.mult, op1=ALU.mult,
                    )
                nc.scalar.dma_start(out=oq[s, t], in_=xts[t])


def build_nc(ns=NS, hw=HW, xbufs=11):
    nc = bacc.Bacc(
        "TRN2", target_bir_lowering=False, debug=False, num_devices=NCORES
    )
    x = nc.dram_tensor("x", [ns, hw, C], f32, kind="ExternalInput").ap()
    r = nc.dram_tensor("r", [ns, C], f32, kind="ExternalInput").ap()
    u = nc.dram_tensor("u", [ns, C], f32, kind="ExternalInput").ap()
    o = nc.dram_tensor("o", [ns, hw, C], f32, kind="ExternalOutput").ap()
    with tile.TileContext(nc) as tc:
        emit(tc, o, x, r, u, ns, hw, xbufs)
    nc.compile()
    return nc


_cached_nc = None


def kernel(x, r, u):
    global _cached_nc
    if _cached_nc is None:
        _cached_nc = build_nc()
    in_maps = [
        {
            "x": np.ascontiguousarray(x[i * NS:(i + 1) * NS], dtype=np.float32),
            "r": np.ascontiguousarray(r[i * NS:(i + 1) * NS], dtype=np.float32),
            "u": np.ascontiguousarray(u[i * NS:(i + 1) * NS], dtype=np.float32),
        }
        for i in range(NCORES)
    ]
    res = run_bass_kernel_spmd(_cached_nc, in_maps, list(range(NCORES))).results
    return np.concatenate([res[i]["o"] for i in range(NCORES)], axis=0)


# revision 19
# speedup vs baseline: 1.1274x; 1.1143x over previous
"""Trainium2 Bass kernel for DropChannel (topk channel masking).

Math (per sample):
    score_c = mean_hw x[hw, c]                       (only sums needed; 1/HW cancels)
    lk_c    = ln(r_c) * (1 / S_c)                    (log of key r**(1/score); order-preserving)
    gcnt_i  = #{c : lk_c > lk_i}                     (strictly-greater count)
    sel_i   = gcnt_i < C - M                         (identical to thr = sort(key)[C-M]; sel = key >= thr,
                                                      including tie behaviour)
    alpha   = sum(S) / sum(S * sel)
    out     = x * (sel & (u < P)) * alpha

Sharding: pure data parallel, N=32 samples -> 8 cores x 4 samples.

Per-core schedule (4 samples, each [4096, 1024] f32):
  - x repacked so each partition holds FOUR consecutive hw rows contiguous
    in HBM -> 16 KiB DMA descriptors (4x fewer packets, measured ~26 B/ns
    per DMA engine vs ~20 at 4 KiB)
  - compute engines get clock-throttled ~1.7-2x while DMA is saturated, so
    engine budgets matter: score sums go to the PE as direct fp32 matmuls
    (8 per chunk) for samples 1-3 (PE ~75us/sample < 97us period; DVE must
    stay under budget for the mask multiplies), while sample 0 uses a
    2-level f-slice DVE tree-add (3 adds + 2 matmuls per chunk) because
    its solo-load phase is too short for 64 fp32 matmuls but leaves DVE
    idle. Rounding stays ~1e-6 either way (selection sits ~1e-5 from the
    boundary)
  - score/lk layout transposes via small DMA hops (32B descriptors, cheap);
    reciprocal only ever runs on the [128, 8] column layout (a
    single-partition [1,C] DVE reciprocal costs ~7.8us)
  - greater-counts: DVE compares of the gpsimd-broadcast lk row (SBUF;
    DVE reads from PSUM cost +60%) against per-partition lk scalars,
    summed across partitions by full-rate bf16 PE ones-matmuls
  - mask rows (0/1, exact in bf16) replicated via gpsimd; in-place multiply
    of cached chunks; loads ride qSP, stores ride qACT; chunk 0's store is
    split in half so the store stream starts two multiplies earlier
"""

import numpy as np
from contextlib import ExitStack

import concourse.bacc as bacc
import concourse.tile as tile
from concourse import mybir
from concourse.bass_utils import run_bass_kernel_spmd

N, HW, C = 32, 4096, 1024
NCORES = 8
NS = N // NCORES          # samples per core
P = 128                   # partitions
CK = C // P               # 8 channels per partition in (k p) layout
NKEEP = C - int(0.5 * C)  # gcnt threshold: keep rows with gcnt < 512
PKEEP = 0.9
HALF = 512                # matmul free-dim limit (one PSUM bank)
FOUR = 4                  # hw rows per partition per chunk (16 KiB descriptors)
CHF = FOUR * C            # chunk free dim (f32 elems per partition)

f32 = mybir.dt.float32
bf16 = mybir.dt.bfloat16
ALU = mybir.AluOpType
ACTF = mybir.ActivationFunctionType
AXIS = mybir.AxisListType


def emit(tc, o, x, r, u, ns, hw, xbufs):
    nc = tc.nc
    nch = hw // (P * FOUR)                # chunks per sample
    xq = x.rearrange("s (q p four) c -> s q p (four c)", p=P, four=FOUR)
    oq = o.rearrange("s (q p four) c -> s q p (four c)", p=P, four=FOUR)
    rck = r.rearrange("s (p k) -> s p k", k=CK)  # channel c = p*8 + k

    with ExitStack() as ctx:
        xpool = ctx.enter_context(tc.tile_pool(name="xpool", bufs=xbufs))
        accpool = ctx.enter_context(tc.tile_pool(name="accpool", bufs=2))
        tqpool = ctx.enter_context(tc.tile_pool(name="tqpool", bufs=2))
        bcpool = ctx.enter_context(tc.tile_pool(name="bcpool", bufs=1))
        rows = ctx.enter_context(tc.tile_pool(name="rows", bufs=1))
        consts = ctx.enter_context(tc.tile_pool(name="consts", bufs=1))
        ps_s = ctx.enter_context(tc.tile_pool(name="ps_s", bufs=1, space="PSUM"))
        ps_g = ctx.enter_context(tc.tile_pool(name="ps_g", bufs=1, space="PSUM"))

        ones_col = consts.tile([P, 1], f32)
        nc.vector.memset(ones_col, 1.0)
        ones_b = consts.tile([P, 1], bf16)
        nc.vector.memset(ones_b, 1.0)

        for s in range(ns):
            # precompute pieces that do not depend on x (overlap with loads)
            lnr_cols = rows.tile([P, CK], f32, tag="lnr_cols")
            nc.scalar.dma_start(out=lnr_cols, in_=rck[s])
            nc.scalar.activation(lnr_cols, lnr_cols, ACTF.Ln)
            rng_row = rows.tile([1, C], f32, tag="rng_row")
            nc.scalar.dma_start(out=rng_row, in_=u[s:s + 1, :])
            nc.vector.tensor_scalar(rng_row, rng_row, PKEEP, None, op0=ALU.is_lt)

            # ---- pass 1: chunk loads + score sums into an accumulating
            # PSUM slot. Sample 0: DVE f-slice tree (PE too slow for its
            # short solo-load phase); samples 1+: direct fp32 matmuls on
            # each f-slice (DVE budget is the scarce resource in steady
            # state once the mask multiplies start overlapping).
            ps_score = ps_s.tile([1, C], f32, tag="ps_score")
            xts = []
            for t in range(nch):
                xtile = xpool.tile([P, CHF], f32, tag="xt")
                # sample 0 has no store traffic yet: split its loads across
                # both HWDGE queues
                if s == 0 and t % 2 == 1:
                    nc.scalar.dma_start(out=xtile, in_=xq[s, t])
                else:
                    nc.sync.dma_start(out=xtile, in_=xq[s, t])
                xts.append(xtile)
                if s == 0:
                    t0 = accpool.tile([P, C], f32, tag="t0", bufs=1)
                    t1 = accpool.tile([P, C], f32, tag="t1", bufs=1)
                    nc.vector.tensor_add(t0, xtile[:, 0:C], xtile[:, C:2 * C])
                    nc.vector.tensor_add(
                        t1, xtile[:, 2 * C:3 * C], xtile[:, 3 * C:]
                    )
                    nc.vector.tensor_add(t0, t0, t1)
                    for h in range(2):
                        nc.tensor.matmul(
                            ps_score[:, h * HALF:(h + 1) * HALF],
                            lhsT=ones_col,
                            rhs=t0[:, h * HALF:(h + 1) * HALF],
                            start=(t == 0),
                            stop=(t == nch - 1),
                        )
                else:
                    for f in range(FOUR):
                        for h in range(2):
                            nc.tensor.matmul(
                                ps_score[:, h * HALF:(h + 1) * HALF],
                                lhsT=ones_col,
                                rhs=xtile[:, f * C + h * HALF:
                                          f * C + (h + 1) * HALF],
                                start=(t == 0 and f == 0),
                                stop=(t == nch - 1 and f == FOUR - 1),
                            )

            # ---- mid: selection mask ----
            # PSUM -> SBUF score row, halves copied by both engines at once
            s_row = rows.tile([1, C], f32, tag="s_row")
            nc.scalar.copy(s_row[:, 0:HALF], ps_score[:, 0:HALF])
            nc.vector.tensor_copy(s_row[:, HALF:], ps_score[:, HALF:])
            # column layout via DMA hop; reciprocal only ever runs on the
            # [128, 8] layout (a [1, C] DVE reciprocal costs ~7.8us)
            s_cols = rows.tile([P, CK], f32, tag="s_cols")
            nc.scalar.dma_start(out=s_cols, in_=s_row)
            # total score sum for alpha (scalar engine, off the DVE critical
            # path; mask_row is pure scratch output here, overwritten below)
            stats = rows.tile([1, 4], f32, tag="stats")
            mask_row = rows.tile([1, C], bf16, tag="mask_row")
            nc.scalar.activation(
                mask_row, s_row, ACTF.Identity, accum_out=stats[:, 1:2]
            )
            recip_cols = rows.tile([P, CK], f32, tag="recip_cols")
            nc.vector.reciprocal(recip_cols, s_cols)
            lk_cols = rows.tile([P, CK], f32, tag="lk_cols")
            nc.vector.tensor_mul(lk_cols, lnr_cols, recip_cols)
            # gcnt_row[i] = #{c : lk_c > lk_i}: DVE compares of the PE-
            # broadcast lk row against per-partition lk scalars, summed
            # across partitions by full-rate bf16 PE ones-matmuls (0/1
            # values are exact in bf16). Pipelined in 512-channel halves.
            lk_row = rows.tile([1, C], f32, tag="lk_row")
            b_bc = bcpool.tile([P, C], f32, tag="b_bc")
            ps_gcnt = ps_g.tile([1, C], f32, tag="ps_gcnt")
            mask_bc = bcpool.tile([P, C], bf16, tag="mask_bc")
            for h in range(2):
                sl = slice(h * HALF, (h + 1) * HALF)
                nc.scalar.dma_start(
                    out=lk_row[:, sl], in_=lk_cols[h * 64:(h + 1) * 64, :]
                )
                nc.gpsimd.partition_broadcast(b_bc[:, sl], lk_row[:, sl])
                for q in range(CK):
                    tq = tqpool.tile([P, HALF], bf16, tag="tq")
                    nc.vector.tensor_scalar(
                        tq, b_bc[:, sl], lk_cols[:, q:q + 1], None, op0=ALU.is_lt
                    )
                    nc.tensor.matmul(
                        ps_gcnt[:, sl],
                        lhsT=ones_b,
                        rhs=tq,
                        start=(q == 0),
                        stop=(q == CK - 1),
                    )
                nc.vector.scalar_tensor_tensor(
                    mask_row[:, sl], ps_gcnt[:, sl], float(NKEEP), rng_row[:, sl],
                    op0=ALU.is_lt, op1=ALU.mult,
                )
                nc.gpsimd.partition_broadcast(mask_bc[:, sl], mask_row[:, sl])

            # alpha = sum(S) / sum(S * sel); rng_row doubles as scratch out
            nc.vector.scalar_tensor_tensor(
                rng_row, ps_gcnt, float(NKEEP), s_row,
                op0=ALU.is_lt, op1=ALU.mult, accum_out=stats[:, 0:1],
            )
            nc.vector.reciprocal(stats[:, 2:3], stats[:, 0:1])
            nc.vector.tensor_scalar(
                stats[:, 2:3], stats[:, 2:3], stats[:, 1:2], None, op0=ALU.mult
            )
            alpha_pp = rows.tile([P, 1], f32, tag="alpha_pp")
            nc.gpsimd.partition_broadcast(alpha_pp, stats[:, 2:3])

            # ---- pass 2: out = (x * alpha) * mask, in place, then store.
            # Chunk 0's store is split in half (8 KiB descriptors) so the
            # store stream starts after two multiplies instead of four.
            for t in range(nch):
                for f in range(FOUR):
                    nc.vector.scalar_tensor_tensor(
                        xts[t][:, f * C:(f + 1) * C],
                        xts[t][:, f * C:(f + 1) * C],
                        alpha_pp, mask_bc,
                        op0=ALU.mult, op1=ALU.mult,
                    )
                    if t == 0 and f == 1:
                        nc.scalar.dma_start(
                            out=oq[s, t, :, 0:2 * C], in_=xts[t][:, 0:2 * C]
                        )
                if t == 0:
                    nc.scalar.dma_start(
                        out=oq[s, t, :, 2 * C:], in_=xts[t][:, 2 * C:]
                    )
                else:
                    nc.scalar.dma_start(out=oq[s, t], in_=xts[t])


def build_nc(ns=NS, hw=HW, xbufs=11):
    nc = bacc.Bacc(
        "TRN2", target_bir_lowering=False, debug=False, num_devices=NCORES
    )
    x = nc.dram_tensor("x", [ns, hw, C], f32, kind="ExternalInput").ap()
    r = nc.dram_tensor("r", [ns, C], f32, kind="ExternalInput").ap()
    u = nc.dram_tensor("u", [ns, C], f32, kind="ExternalInput").ap()
    o = nc.dram_tensor("o", [ns, hw, C], f32, kind="ExternalOutput").ap()
    with tile.TileContext(nc) as tc:
        emit(tc, o, x, r, u, ns, hw, xbufs)
    nc.compile()
    return nc


_cached_nc = None


def kernel(x, r, u):
    global _cached_nc
    if _cached_nc is None:
        _cached_nc = build_nc()
    in_maps = [
        {
            "x": np.ascontiguousarray(x[i * NS:(i + 1) * NS], dtype=np.float32),
            "r": np.ascontiguousarray(r[i * NS:(i + 1) * NS], dtype=np.float32),
            "u": np.ascontiguousarray(u[i * NS:(i + 1) * NS], dtype=np.float32),
        }
        for i in range(NCORES)
    ]
    res = run_bass_kernel_spmd(_cached_nc, in_maps, list(range(NCORES))).results
    return np.concatenate([res[i]["o"] for i in range(NCORES)], axis=0)


# revision 23
# speedup vs baseline: 1.2407x; 1.1005x over previous
"""Trainium2 Bass kernel for DropChannel (topk channel masking).

Math (per sample):
    score_c = mean_hw x[hw, c]                       (only sums needed; 1/HW cancels)
    lk_c    = ln(r_c) * (1 / S_c)                    (log of key r**(1/score); order-preserving)
    gcnt_i  = #{c : lk_c > lk_i}                     (strictly-greater count)
    sel_i   = gcnt_i < C - M                         (identical to thr = sort(key)[C-M]; sel = key >= thr,
                                                      including tie behaviour)
    alpha   = sum(S) / sum(S * sel)
    out     = x * (sel & (u < P)) * alpha

Sharding: pure data parallel, N=32 samples -> 8 cores x 4 samples.

Per-core schedule (4 samples, each [4096, 1024] f32):
  - x repacked so each partition holds FOUR consecutive hw rows contiguous
    in HBM -> 16 KiB DMA descriptors (4x fewer packets, measured ~26 B/ns
    per DMA engine vs ~20 at 4 KiB)
  - compute engines get clock-throttled ~1.7-2x while DMA is saturated, so
    engine budgets matter: score sums go to the PE as direct fp32 matmuls
    (8 per chunk) for samples 1-3 (PE ~75us/sample < 97us period; DVE must
    stay under budget for the mask multiplies), while sample 0 uses a
    2-level f-slice DVE tree-add (3 adds + 2 matmuls per chunk) because
    its solo-load phase is too short for 64 fp32 matmuls but leaves DVE
    idle. Rounding stays ~1e-6 either way (selection sits ~1e-5 from the
    boundary)
  - score/lk layout transposes via small DMA hops (32B descriptors, cheap);
    reciprocal only ever runs on the [128, 8] column layout (a
    single-partition [1,C] DVE reciprocal costs ~7.8us)
  - greater-counts: DVE compares of the gpsimd-broadcast lk row (SBUF;
    DVE reads from PSUM cost +60%) against per-partition lk scalars,
    summed across partitions by full-rate bf16 PE ones-matmuls
  - mask rows (0/1, exact in bf16) replicated via gpsimd; in-place multiply
    of cached chunks; loads ride qSP, stores ride qACT; chunk 0's store is
    split in half so the store stream starts two multiplies earlier
"""

import numpy as np
from contextlib import ExitStack

import concourse.bacc as bacc
import concourse.tile as tile
from concourse import mybir
from concourse.bass_utils import run_bass_kernel_spmd

N, HW, C = 32, 4096, 1024
NCORES = 8
NS = N // NCORES          # samples per core
P = 128                   # partitions
CK = C // P               # 8 channels per partition in (k p) layout
NKEEP = C - int(0.5 * C)  # gcnt threshold: keep rows with gcnt < 512
PKEEP = 0.9
HALF = 512                # matmul free-dim limit (one PSUM bank)
FOUR = 4                  # hw rows per partition per chunk (16 KiB descriptors)
CHF = FOUR * C            # chunk free dim (f32 elems per partition)

f32 = mybir.dt.float32
bf16 = mybir.dt.bfloat16
ALU = mybir.AluOpType
ACTF = mybir.ActivationFunctionType
AXIS = mybir.AxisListType


def emit(tc, o, x, r, u, ns, hw, xbufs):
    nc = tc.nc
    nch = hw // (P * FOUR)                # chunks per sample
    xq = x.rearrange("s (q p four) c -> s q p (four c)", p=P, four=FOUR)
    oq = o.rearrange("s (q p four) c -> s q p (four c)", p=P, four=FOUR)
    rck = r.rearrange("s (p k) -> s p k", k=CK)  # channel c = p*8 + k

    with ExitStack() as ctx:
        xpool = ctx.enter_context(tc.tile_pool(name="xpool", bufs=xbufs))
        accpool = ctx.enter_context(tc.tile_pool(name="accpool", bufs=1))
        tqpool = ctx.enter_context(tc.tile_pool(name="tqpool", bufs=4))
        bcpool = ctx.enter_context(tc.tile_pool(name="bcpool", bufs=1))
        rows = ctx.enter_context(tc.tile_pool(name="rows", bufs=1))
        consts = ctx.enter_context(tc.tile_pool(name="consts", bufs=1))
        ps_s = ctx.enter_context(tc.tile_pool(name="ps_s", bufs=1, space="PSUM"))
        ps_g = ctx.enter_context(tc.tile_pool(name="ps_g", bufs=1, space="PSUM"))

        ones_col = consts.tile([P, 1], f32)
        nc.vector.memset(ones_col, 1.0)
        ones_b = consts.tile([P, 1], bf16)
        nc.vector.memset(ones_b, 1.0)

        # The TRN2 PE ramps its clock with sustained use (p-state: 0.65 GHz
        # cold -> 1.2+ GHz warm; cold matmuls measure ~1.8x slower). Warm it
        # up during sample 0's load phase with dummy matmuls.
        warm = consts.tile([P, HALF], bf16)
        nc.vector.memset(warm, 1.0)
        ps_warm = ps_g.tile([1, HALF], f32, tag="warm")
        for w in range(16):
            nc.tensor.matmul(
                ps_warm, lhsT=ones_b, rhs=warm, start=(w == 0), stop=(w == 15)
            )

        for s in range(ns):
            # precompute pieces that do not depend on x (overlap with loads)
            lnr_cols = rows.tile([P, CK], f32, tag="lnr_cols")
            nc.scalar.dma_start(out=lnr_cols, in_=rck[s])
            nc.scalar.activation(lnr_cols, lnr_cols, ACTF.Ln)
            rng_row = rows.tile([1, C], f32, tag="rng_row")
            nc.scalar.dma_start(out=rng_row, in_=u[s:s + 1, :])
            nc.vector.tensor_scalar(rng_row, rng_row, PKEEP, None, op0=ALU.is_lt)

            # ---- pass 1: chunk loads + score sums into an accumulating
            # PSUM slot. Sample 0: DVE f-slice tree (PE too slow for its
            # short solo-load phase); samples 1+: direct fp32 matmuls on
            # each f-slice (DVE budget is the scarce resource in steady
            # state once the mask multiplies start overlapping).
            ps_score = ps_s.tile([1, C], f32, tag="ps_score")
            xts = []
            for t in range(nch):
                xtile = xpool.tile([P, CHF], f32, tag="xt")
                # sample 0 has no store traffic yet: split its loads across
                # both HWDGE queues
                if s == 0 and t % 2 == 1:
                    nc.scalar.dma_start(out=xtile, in_=xq[s, t])
                else:
                    nc.sync.dma_start(out=xtile, in_=xq[s, t])
                xts.append(xtile)
                if s == 0:
                    t0 = accpool.tile([P, C], f32, tag="t0", bufs=1)
                    nc.vector.tensor_add(t0, xtile[:, 0:C], xtile[:, C:2 * C])
                    nc.vector.tensor_add(t0, t0, xtile[:, 2 * C:3 * C])
                    nc.vector.tensor_add(t0, t0, xtile[:, 3 * C:])
                    for h in range(2):
                        nc.tensor.matmul(
                            ps_score[:, h * HALF:(h + 1) * HALF],
                            lhsT=ones_col,
                            rhs=t0[:, h * HALF:(h + 1) * HALF],
                            start=(t == 0),
                            stop=(t == nch - 1),
                        )
                else:
                    for f in range(FOUR):
                        for h in range(2):
                            nc.tensor.matmul(
                                ps_score[:, h * HALF:(h + 1) * HALF],
                                lhsT=ones_col,
                                rhs=xtile[:, f * C + h * HALF:
                                          f * C + (h + 1) * HALF],
                                start=(t == 0 and f == 0),
                                stop=(t == nch - 1 and f == FOUR - 1),
                            )

            # ---- mid: selection mask ----
            # PSUM -> SBUF score row, halves copied by both engines at once
            s_row = rows.tile([1, C], f32, tag="s_row")
            nc.scalar.copy(s_row[:, 0:HALF], ps_score[:, 0:HALF])
            nc.vector.tensor_copy(s_row[:, HALF:], ps_score[:, HALF:])
            # column layout via DMA hop; reciprocal only ever runs on the
            # [128, 8] layout (a [1, C] DVE reciprocal costs ~7.8us)
            s_cols = rows.tile([P, CK], f32, tag="s_cols")
            nc.scalar.dma_start(out=s_cols, in_=s_row)
            # total score sum for alpha (scalar engine, off the DVE critical
            # path; mask_row is pure scratch output here, overwritten below)
            stats = rows.tile([1, 4], f32, tag="stats")
            mask_row = rows.tile([1, C], bf16, tag="mask_row")
            nc.scalar.activation(
                mask_row, s_row, ACTF.Identity, accum_out=stats[:, 1:2]
            )
            recip_cols = rows.tile([P, CK], f32, tag="recip_cols")
            nc.vector.reciprocal(recip_cols, s_cols)
            lk_cols = rows.tile([P, CK], f32, tag="lk_cols")
            nc.vector.tensor_mul(lk_cols, lnr_cols, recip_cols)
            # gcnt_row[i] = #{c : lk_c > lk_i}: DVE compares of the PE-
            # broadcast lk row against per-partition lk scalars, summed
            # across partitions by full-rate bf16 PE ones-matmuls (0/1
            # values are exact in bf16). Pipelined in 512-channel halves.
            lk_row = rows.tile([1, C], f32, tag="lk_row")
            b_bc = bcpool.tile([P, C], f32, tag="b_bc")
            ps_gcnt = ps_g.tile([1, C], f32, tag="ps_gcnt")
            mask_bc = bcpool.tile([P, C], bf16, tag="mask_bc")
            for h in range(2):
                sl = slice(h * HALF, (h + 1) * HALF)
                nc.scalar.dma_start(
                    out=lk_row[:, sl], in_=lk_cols[h * 64:(h + 1) * 64, :]
                )
                nc.gpsimd.partition_broadcast(b_bc[:, sl], lk_row[:, sl])
                for q in range(CK):
                    tq = tqpool.tile([P, HALF], bf16, tag="tq")
                    nc.vector.tensor_scalar(
                        tq, b_bc[:, sl], lk_cols[:, q:q + 1], None, op0=ALU.is_lt
                    )
                    nc.tensor.matmul(
                        ps_gcnt[:, sl],
                        lhsT=ones_b,
                        rhs=tq,
                        start=(q == 0),
                        stop=(q == CK - 1),
                    )
                nc.vector.scalar_tensor_tensor(
                    mask_row[:, sl], ps_gcnt[:, sl], float(NKEEP), rng_row[:, sl],
                    op0=ALU.is_lt, op1=ALU.mult,
                )
                nc.gpsimd.partition_broadcast(mask_bc[:, sl], mask_row[:, sl])

            # alpha = sum(S) / sum(S * sel); rng_row doubles as scratch out
            nc.vector.scalar_tensor_tensor(
                rng_row, ps_gcnt, float(NKEEP), s_row,
                op0=ALU.is_lt, op1=ALU.mult, accum_out=stats[:, 0:1],
            )
            nc.vector.reciprocal(stats[:, 2:3], stats[:, 0:1])
            nc.vector.tensor_scalar(
                stats[:, 2:3], stats[:, 2:3], stats[:, 1:2], None, op0=ALU.mult
            )
            alpha_pp = rows.tile([P, 1], f32, tag="alpha_pp")
            nc.gpsimd.partition_broadcast(alpha_pp, stats[:, 2:3])

            # ---- pass 2: out = (x * alpha) * mask, in place, then store.
            # Chunk 0's store is split in half (8 KiB descriptors) so the
            # store stream starts after two multiplies instead of four.
            for t in range(nch):
                for f in range(FOUR):
                    nc.vector.scalar_tensor_tensor(
                        xts[t][:, f * C:(f + 1) * C],
                        xts[t][:, f * C:(f + 1) * C],
                        alpha_pp, mask_bc,
                        op0=ALU.mult, op1=ALU.mult,
                    )
                    if t == 0 and f < 2:
                        # chunk 0 streams out per f-slice so the store
                        # queue starts one multiply after alpha
                        nc.scalar.dma_start(
                            out=oq[s, t, :, f * C:(f + 1) * C],
                            in_=xts[t][:, f * C:(f + 1) * C],
                        )
                if t == 0:
                    nc.scalar.dma_start(
                        out=oq[s, t, :, 2 * C:], in_=xts[t][:, 2 * C:]
                    )
                else:
                    nc.scalar.dma_start(out=oq[s, t], in_=xts[t])


def build_nc(ns=NS, hw=HW, xbufs=11):
    nc = bacc.Bacc(
        "TRN2", target_bir_lowering=False, debug=False, num_devices=NCORES
    )
    x = nc.dram_tensor("x", [ns, hw, C], f32, kind="ExternalInput").ap()
    r = nc.dram_tensor("r", [ns, C], f32, kind="ExternalInput").ap()
    u = nc.dram_tensor("u", [ns, C], f32, kind="ExternalInput").ap()
    o = nc.dram_tensor("o", [ns, hw, C], f32, kind="ExternalOutput").ap()
    with tile.TileContext(nc) as tc:
        emit(tc, o, x, r, u, ns, hw, xbufs)
    nc.compile()
    return nc


_cached_nc = None


def kernel(x, r, u):
    global _cached_nc
    if _cached_nc is None:
        _cached_nc = build_nc()
    in_maps = [
        {
            "x": np.ascontiguousarray(x[i * NS:(i + 1) * NS], dtype=np.float32),
            "r": np.ascontiguousarray(r[i * NS:(i + 1) * NS], dtype=np.float32),
            "u": np.ascontiguousarray(u[i * NS:(i + 1) * NS], dtype=np.float32),
        }
        for i in range(NCORES)
    ]
    res = run_bass_kernel_spmd(_cached_nc, in_maps, list(range(NCORES))).results
    return np.concatenate([res[i]["o"] for i in range(NCORES)], axis=0)
